# revision 1
# baseline (speedup 1.0000x reference)
"""Causal self-attention TRN2 kernel: 8-way head-parallel (2 heads/core).

Layout strategy (per core c, heads h0=2c, h1=2c+1):
  - Host pre-transposes x -> xT [1024, 4096] (tokens b-major) and slices/permutes
    weights so q/k head dims are [32 evens | 32 odds] (de-interleaved RoPE).
  - QKV projection computes qT/kT/vT [128 (2 heads' dims), tok] via f32r
    matmuls accumulating over 8 c-chunks, in 1024-col blocks.
  - RoPE on qT/kT with partition-aligned ops (signed sin table + 32-row swap).
  - V transposed per 128-tok chunk into V_aug [tok 128, V(64) | ones(64)] so the
    AV matmul also produces the softmax denominator in psum rows 64:127.
  - Scores computed TRANSPOSED: S^T[k,q] chunks, k-chunks batched in pairs so
    one ACT exp covers 2 chunks (scale=1/8; |s|<~20 so no max subtraction);
    causal mask via gpsimd affine_select (multiplicative zero post-exp).
  - Single PSUM pool, phases interleaved per batch so attention/out-proj of
    batch 0 overlap QKV of batch 1.
  - out-proj: lhsT=outT chunk [128,128], rhs=woT [128,1024] -> partial [tok,1024]
  - Host sums 8 partials (the tensor-parallel all-reduce) and reshapes.
"""

import sys

if "/opt/trn_rl_repo" not in sys.path:
    sys.path.insert(0, "/opt/trn_rl_repo")

import numpy as np

import concourse.bass as bass
import concourse.tile as tile
from concourse import bacc, mybir
from concourse.masks import make_identity

F32 = mybir.dt.float32
F32R = mybir.dt.float32r
EXP = mybir.ActivationFunctionType.Exp

B, T, D, H, DH = 2, 2048, 1024, 16, 64
NCORES = 8
TOK = B * T          # 4096
QB = 512             # attention q-block (one psum bank wide)
KC = 128             # k chunk
EG = 2               # exp batch: k-chunks per ACT exp
NKC = T // KC        # 16 k-chunks per unit
NQB = T // QB        # 4 q-blocks per unit
PB = 1024            # QKV/out-proj column block
CPJ = D // 128       # 8 contraction chunks


def build_program():
    nc = bacc.Bacc("TRN2", target_bir_lowering=False, debug=False,
                   num_devices=NCORES)
    xT = nc.dram_tensor("xT", [D, TOK], F32R, kind="ExternalInput").ap()
    wq = nc.dram_tensor("wq", [D, 128], F32R, kind="ExternalInput").ap()
    wk = nc.dram_tensor("wk", [D, 128], F32R, kind="ExternalInput").ap()
    wv = nc.dram_tensor("wv", [D, 128], F32R, kind="ExternalInput").ap()
    woT = nc.dram_tensor("woT", [128, D], F32R, kind="ExternalInput").ap()
    cosT = nc.dram_tensor("cosT", [32, T], F32, kind="ExternalInput").ap()
    sinTp = nc.dram_tensor("sinTp", [32, T], F32, kind="ExternalInput").ap()
    sinTn = nc.dram_tensor("sinTn", [32, T], F32, kind="ExternalInput").ap()
    partial = nc.dram_tensor("partial", [TOK, D], F32, kind="ExternalOutput").ap()

    with tile.TileContext(nc) as tc:
        with tc.tile_pool(name="sb", bufs=1) as sb, \
             tc.tile_pool(name="ps", bufs=1, space="PSUM") as ps:
            # persistent SBUF tiles
            wq_sb = sb.tile([128, CPJ, 128], F32R, name="wq_sb", tag="wq_sb")
            wk_sb = sb.tile([128, CPJ, 128], F32R, name="wk_sb", tag="wk_sb")
            wv_sb = sb.tile([128, CPJ, 128], F32R, name="wv_sb", tag="wv_sb")
            woT_sb = sb.tile([128, D], F32R, name="woT_sb", tag="woT_sb")
            cos_full = sb.tile([128, T], F32, name="cos_full", tag="cos_full")
            sin_full = sb.tile([128, T], F32, name="sin_full", tag="sin_full")
            ident = sb.tile([64, 64], F32, name="ident", tag="ident")
            identB = sb.tile([128, 64], F32, name="identB", tag="identB")
            ones64 = sb.tile([128, 64], F32, name="ones64", tag="ones64")
            qT2 = sb.tile([128, TOK], F32R, name="qT2", tag="qT2")
            kT2 = sb.tile([128, TOK], F32R, name="kT2", tag="kT2")
            outT = sb.tile([128, TOK], F32R, name="outT", tag="outT")
            vaug = [sb.tile([128, NKC, 128], F32R, name=f"vaug{u}", tag=f"vaug{u}")
                    for u in range(4)]

            def emit_setup():
                make_identity(nc, ident[:], nomemset=False)
                nc.gpsimd.memset(identB[:], 0.0)
                nc.gpsimd.affine_select(out=identB[:], in_=identB[:],
                                        compare_op=mybir.AluOpType.not_equal,
                                        fill=1.0, base=-64, pattern=[[-1, 64]],
                                        channel_multiplier=1)
                nc.gpsimd.memset(ones64[:], 1.0)
                for blk in range(4):
                    nc.sync.dma_start(out=cos_full[blk * 32:(blk + 1) * 32, :],
                                      in_=cosT[:])
                nc.sync.dma_start(out=sin_full[0:32, :], in_=sinTp[:])
                nc.sync.dma_start(out=sin_full[32:64, :], in_=sinTn[:])
                nc.sync.dma_start(out=sin_full[64:96, :], in_=sinTp[:])
                nc.sync.dma_start(out=sin_full[96:128, :], in_=sinTn[:])
                nc.sync.dma_start(out=woT_sb[:], in_=woT[:])

            def emit_qkv_block(s):
                scol = s * PB
                tcol = (s % (T // PB)) * PB
                b = s // (T // PB)
                pss = {}
                for nm in ("q", "k", "v"):
                    pss[nm] = ps.tile([128, PB], F32, name=f"{nm}ps{s}",
                                      tag="big", bufs=3)
                for j in range(CPJ):
                    xts = sb.tile([128, PB], F32R, name=f"xts{s}_{j}",
                                  tag="xts", bufs=3)
                    nc.sync.dma_start(
                        out=xts[:], in_=xT[j * 128:(j + 1) * 128, scol:scol + PB])
                    if s == 0:      # interleave weight loads with first block
                        nc.sync.dma_start(out=wq_sb[:, j, :],
                                          in_=wq[j * 128:(j + 1) * 128, :])
                        nc.sync.dma_start(out=wk_sb[:, j, :],
                                          in_=wk[j * 128:(j + 1) * 128, :])
                        nc.sync.dma_start(out=wv_sb[:, j, :],
                                          in_=wv[j * 128:(j + 1) * 128, :])
                    st, sp = (j == 0), (j == CPJ - 1)
                    for nm, wsb in (("q", wq_sb), ("k", wk_sb), ("v", wv_sb)):
                        for hf in range(2):
                            nc.tensor.matmul(
                                pss[nm][:, hf * 512:(hf + 1) * 512],
                                wsb[:, j, :], xts[:, hf * 512:(hf + 1) * 512],
                                start=st, stop=sp)
                if s == 0:
                    emit_setup()

                raws = {}
                for nm in ("q", "k", "v"):
                    raw = sb.tile([128, PB], F32, name=f"raw{nm}{s}",
                                  tag=f"raw{nm}", bufs=2)
                    nc.scalar.copy(raw[:], pss[nm][:])
                    raws[nm] = raw
                for nm, dst in (("q", qT2), ("k", kT2)):
                    raw = raws[nm]
                    ra = sb.tile([128, PB], F32, name=f"ra{nm}{s}", tag="ropeA",
                                 bufs=2)
                    rs = sb.tile([128, PB], F32, name=f"rs{nm}{s}", tag="ropeS",
                                 bufs=2)
                    rw = sb.tile([128, PB], F32, name=f"rw{nm}{s}", tag="ropeW",
                                 bufs=2)
                    nc.vector.tensor_mul(ra[:], raw[:], cos_full[:, tcol:tcol + PB])
                    nc.vector.tensor_mul(rs[:], raw[:], sin_full[:, tcol:tcol + PB])
                    for blk in range(4):
                        src = (blk ^ 1) * 32
                        nc.sync.dma_start(out=rw[blk * 32:(blk + 1) * 32, :],
                                          in_=rs[src:src + 32, :])
                    nc.vector.tensor_add(dst[:, scol:scol + PB], ra[:], rw[:])

                vraw = raws["v"]
                for tp2 in range(PB // KC // 2):    # pairs of 128-tok chunks
                    for h in range(2):
                        u = b * 2 + h
                        ck0 = (s % (T // PB)) * (PB // KC) + 2 * tp2
                        tp = ps.tile([128, 2, 64], F32, name=f"tp{s}_{tp2}_{h}",
                                     tag="avtp", bufs=2)
                        idt = ident[:] if h == 0 else identB[64:128, :]
                        for pi in range(2):
                            tch = 2 * tp2 + pi
                            nc.tensor.transpose(
                                tp[:, pi, :],
                                vraw[h * 64:(h + 1) * 64, tch * KC:(tch + 1) * KC],
                                idt)
                        nc.vector.tensor_copy(vaug[u][:, ck0:ck0 + 2, 0:64], tp[:])

            def emit_attention_unit(u, op_after=False):
                b, h = u // 2, u % 2
                hr = h * 64
                tb = b * T
                for s4 in range(NQB):
                    qc = tb + s4 * QB
                    av = ps.tile([128, QB], F32, name=f"av{u}_{s4}", tag="avtp",
                                 bufs=2)
                    njc = (s4 + 1) * (QB // KC)
                    jgs = [list(range(g, min(g + EG, njc)))
                           for g in range(0, njc, EG)]
                    for jg in jgs:
                        ng = len(jg)
                        sps = ps.tile([128, EG, QB], F32,
                                      name=f"sps{u}_{s4}_{jg[0]}", tag="big",
                                      bufs=3)
                        for gi, j in enumerate(jg):
                            kc = tb + j * KC
                            nc.tensor.matmul(
                                sps[:, gi, :], kT2[hr:hr + 64, kc:kc + KC],
                                qT2[hr:hr + 64, qc:qc + QB],
                                start=True, stop=True)
                        pT = sb.tile([128, EG, QB], F32R,
                                     name=f"pT{u}_{s4}_{jg[0]}", tag="pT",
                                     bufs=4)
                        nc.scalar.activation(pT[:, 0:ng, :], sps[:, 0:ng, :],
                                             EXP, scale=0.125)
                        for gi, j in enumerate(jg):
                            off = j * KC - s4 * QB
                            if off >= -KC + 1:
                                nc.gpsimd.affine_select(
                                    out=pT[:, gi, :], in_=pT[:, gi, :],
                                    compare_op=mybir.AluOpType.is_ge,
                                    fill=0.0, base=-off, pattern=[[1, QB]],
                                    channel_multiplier=-1)
                        for gi, j in enumerate(jg):
                            nc.tensor.matmul(av[:], vaug[u][:, j, :],
                                             pT[:, gi, :], start=(j == 0),
                                             stop=(j == njc - 1))
                    rD = sb.tile([64, QB], F32, name=f"rD{u}_{s4}", tag="rD",
                                 bufs=2)
                    nc.vector.reciprocal(rD[:], av[64:128, :])
                    nc.vector.tensor_mul(outT[hr:hr + 64, qc:qc + QB],
                                         av[0:64, :], rD[:])
                    if op_after:
                        for mm in range(s4 * (QB // 128), (s4 + 1) * (QB // 128)):
                            emit_outproj_tile(b, mm)

            def emit_outproj_batch(b):
                for mm in range(T // 128):
                    emit_outproj_tile(b, mm)

            def emit_outproj_tile(b, mm):
                    col = b * T + mm * 128
                    ops = ps.tile([128, D], F32, name=f"ops{b}_{mm}", tag="big",
                                  bufs=3)
                    for hf in range(2):
                        nc.tensor.matmul(ops[:, hf * 512:(hf + 1) * 512],
                                         outT[:, col:col + 128],
                                         woT_sb[:, hf * 512:(hf + 1) * 512],
                                         start=True, stop=True)
                    osb = sb.tile([128, D], F32, name=f"osb{b}_{mm}", tag="osb",
                                  bufs=3)
                    nc.vector.tensor_copy(osb[:], ops[:])
                    nc.sync.dma_start(out=partial[col:col + 128, :], in_=osb[:])

            # interleaved emission: batch 0 attention overlaps batch 1 QKV
            emit_qkv_block(0)
            emit_qkv_block(1)
            for u in range(4):
                for ck in range(NKC):
                    nc.gpsimd.tensor_copy(out=vaug[u][:, ck, 64:128],
                                          in_=ones64[:])
            emit_attention_unit(0)
            emit_qkv_block(2)
            emit_attention_unit(1)
            emit_qkv_block(3)
            emit_attention_unit(2)
            emit_outproj_batch(0)
            emit_attention_unit(3)
            emit_outproj_batch(1)

    nc.compile()
    return nc


def prep_in_maps(x, rope_freqs, w_qkv, w_out):
    x = np.ascontiguousarray(x, dtype=np.float32)
    w_qkv = np.ascontiguousarray(w_qkv, dtype=np.float32)
    w_out = np.ascontiguousarray(w_out, dtype=np.float32)
    ang = np.asarray(rope_freqs, dtype=np.float64)
    cosT = np.ascontiguousarray(np.cos(ang).T.astype(np.float32))
    sinT = np.ascontiguousarray(np.sin(ang).T.astype(np.float32))
    sinTn = np.ascontiguousarray(-sinT)
    xT = np.ascontiguousarray(x.reshape(TOK, D).T)

    perm64 = np.concatenate([np.arange(0, DH, 2), np.arange(1, DH, 2)])
    in_maps = []
    for c in range(NCORES):
        h0, h1 = 2 * c, 2 * c + 1
        qk_rows = np.concatenate([h0 * DH + perm64, h1 * DH + perm64])
        v_rows = np.arange(h0 * DH, h0 * DH + 2 * DH)
        in_maps.append({
            "xT": xT,
            "wq": np.ascontiguousarray(w_qkv[qk_rows, :].T),
            "wk": np.ascontiguousarray(w_qkv[D + qk_rows, :].T),
            "wv": np.ascontiguousarray(w_qkv[2 * D + v_rows, :].T),
            "woT": np.ascontiguousarray(w_out[:, v_rows].T),
            "cosT": cosT, "sinTp": sinT, "sinTn": sinTn,
        })
    return in_maps


_CACHED = {}


def kernel(x, rope_freqs, w_qkv, w_out):
    from concourse.bass_utils import run_bass_kernel_spmd
    if "nc" not in _CACHED:
        _CACHED["nc"] = build_program()
    nc = _CACHED["nc"]
    in_maps = prep_in_maps(x, rope_freqs, w_qkv, w_out)
    res = run_bass_kernel_spmd(nc, in_maps, list(range(NCORES)))
    acc = np.zeros((TOK, D), dtype=np.float32)
    for r in res.results:
        acc += r["partial"]
    return acc.reshape(B, T, D)



# revision 5
# speedup vs baseline: 1.6963x; 1.6963x over previous
"""Causal self-attention TRN2 kernel v2: 8-way head-parallel, fp8 DoubleRow.

Per core c (heads h0=2c, h1=2c+1):
  - x, w_qkv (x16), w_out (x16) quantized to fp8e4m3 host-side; QKV projection
    runs fp8 DoubleRow matmuls (K=256/instr, 0.5 cyc/row).
  - RoPE pairs laid out [16 evens | 16 odds] per 32-partition quadrant so the
    complex-pair swap is a single DVE stream_shuffle (mask i^16).
  - Scores S^T[k,q] in bf16 (K=64); exp on ACT writes fp8 pT pairs; AV uses
    fp8 DoubleRow over chunk pairs with V_aug [v|ones] giving the softmax
    denominator in psum rows 64:127.
  - Causal: PE fills -1e7 left of the diagonal via K=1 matmuls; gpsimd
    affine_select triangles only the [128,128] diagonal block of pT.
  - out-proj: fp8 DR with outT_dr [64, 2(head), tok]; psum->sbuf bf16 copies
    split across DVE; batched bf16 DMA out (4 tok-tiles per DMA).
  - Host: sum 8 partials, scale 1/256, reshape.
"""

import sys

if "/opt/trn_rl_repo" not in sys.path:
    sys.path.insert(0, "/opt/trn_rl_repo")

import numpy as np

import concourse.bass as bass
import concourse.tile as tile
from concourse import bacc, mybir

F32 = mybir.dt.float32
BF16 = mybir.dt.bfloat16
FP8 = mybir.dt.float8e4
EXP = mybir.ActivationFunctionType.Exp
DR = mybir.MatmulPerfMode.DoubleRow

B, T, D, H, DH = 2, 2048, 1024, 16, 64
NCORES = 8
TOK = B * T
QB = 512
KC = 128
NQB = T // QB          # 4 q-blocks per unit
WSCALE = 16.0          # host scale on w_qkv / w_out before fp8
EXPSCALE = 0.125 / (WSCALE * WSCALE)
SHUF_MASK = [i ^ 16 for i in range(32)]


def build_program():
    nc = bacc.Bacc("TRN2", target_bir_lowering=False, debug=False,
                   num_devices=NCORES)
    xdr = nc.dram_tensor("xdr", [128, 8, TOK], FP8, kind="ExternalInput").ap()
    wall = nc.dram_tensor("wall", [128, 8, 384], FP8, kind="ExternalInput").ap()
    wodr = nc.dram_tensor("wodr", [64, 2, D], FP8, kind="ExternalInput").ap()
    cosd = nc.dram_tensor("cosd", [128, T], BF16, kind="ExternalInput").ap()
    sind = nc.dram_tensor("sind", [128, T], BF16, kind="ExternalInput").ap()
    partial = nc.dram_tensor("partial", [128, 32, D], BF16,
                             kind="ExternalOutput").ap()

    with tile.TileContext(nc) as tc:
        with tc.tile_pool(name="sb", bufs=1) as sb, \
             tc.tile_pool(name="ps", bufs=1, space="PSUM") as ps:
            # persistent SBUF
            w_sb = sb.tile([128, 8, 384], FP8, name="w_sb", tag="w_sb")
            wo_sb = sb.tile([64, 2, D], FP8, name="wo_sb", tag="wo_sb")
            cos_sb = sb.tile([128, T], BF16, name="cos_sb", tag="cos_sb")
            sin_sb = sb.tile([128, T], BF16, name="sin_sb", tag="sin_sb")
            qT2 = sb.tile([128, TOK], BF16, name="qT2", tag="qT2")
            kT2 = sb.tile([128, TOK], BF16, name="kT2", tag="kT2")
            outT = sb.tile([64, 2, TOK], FP8, name="outT", tag="outT")
            vaug = [sb.tile([128, 16, 128], FP8, name=f"vaug{u}", tag=f"vaug{u}")
                    for u in range(4)]
            identB = sb.tile([128, 64], BF16, name="identB", tag="identB")
            negrow = sb.tile([1, 512], BF16, name="negrow", tag="negrow")
            onesrow = sb.tile([1, 512], BF16, name="onesrow", tag="onesrow")

            def emit_setup():
                nc.sync.dma_start(out=w_sb[:], in_=wall[:])
                nc.sync.dma_start(out=wo_sb[:], in_=wodr[:])
                nc.sync.dma_start(out=cos_sb[:], in_=cosd[:])
                nc.sync.dma_start(out=sin_sb[:], in_=sind[:])
                nc.gpsimd.memset(identB[:], 0.0)
                nc.gpsimd.affine_select(out=identB[:], in_=identB[:],
                                        compare_op=mybir.AluOpType.not_equal,
                                        fill=1.0, base=-64, pattern=[[-1, 64]],
                                        channel_multiplier=1)
                nc.gpsimd.memset(negrow[:], -1.0e7)
                nc.gpsimd.memset(onesrow[:], 1.0)
                for u in range(4):
                    nc.gpsimd.memset(vaug[u][:, :, 64:128], 1.0)

            def emit_qkv_block(s):
                # block s: tokens [s*1024, (s+1)*1024); batch b = s // 2
                scol = s * 1024
                tcol = (s % 2) * 1024
                b = s // 2
                xts = sb.tile([128, 8, 1024], FP8, name=f"xts{s}", tag="xts",
                              bufs=2)
                nc.sync.dma_start(out=xts[:], in_=xdr[:, :, scol:scol + 1024])
                if s == 0:
                    emit_setup()
                for t, nm in ((0, "q"), (1, "k"), (2, "v")):
                    pp = ps.tile([128, 1024], F32, name=f"{nm}ps{s}",
                                 tag="big", bufs=1)
                    for hf in range(2):
                        for jj in range(4):
                            nc.tensor.matmul(
                                pp[:, hf * 512:(hf + 1) * 512],
                                w_sb[:, 2 * jj:2 * jj + 2,
                                     t * 128:(t + 1) * 128],
                                xts[:, 2 * jj:2 * jj + 2,
                                    hf * 512:(hf + 1) * 512],
                                start=(jj == 0), stop=(jj == 3), perf_mode=DR)
                    if nm == "v":
                        vraw = sb.tile([128, 1024], BF16, name=f"vraw{s}",
                                       tag="vraw", bufs=2)
                        nc.scalar.copy(vraw[:], pp[:])
                        for h in range(2):
                            u = b * 2 + h
                            idt = identB[0:64, :] if h == 0 else identB[64:128, :]
                            for g in range(2):
                                tp = ps.tile([128, 4, 64], BF16,
                                             name=f"tp{s}_{h}_{g}", tag="av",
                                             bufs=2)
                                for pi in range(4):
                                    tch = g * 4 + pi
                                    nc.tensor.transpose(
                                        tp[:, pi, :],
                                        vraw[h * 64:(h + 1) * 64,
                                             tch * 128:(tch + 1) * 128],
                                        idt)
                                cj = (s % 2) * 8 + g * 4
                                nc.scalar.copy(
                                    vaug[u][:, cj:cj + 4, 0:64], tp[:])
                    else:
                        dst = qT2 if nm == "q" else kT2
                        ra = sb.tile([128, 1024], BF16, name=f"ra{nm}{s}",
                                     tag="ra", bufs=2)
                        rs = sb.tile([128, 1024], BF16, name=f"rs{nm}{s}",
                                     tag="rs", bufs=2)
                        rw = sb.tile([128, 1024], BF16, name=f"rw{nm}{s}",
                                     tag="rw", bufs=2)
                        nc.vector.tensor_mul(ra[:], pp[:],
                                             cos_sb[:, tcol:tcol + 1024])
                        nc.vector.tensor_mul(rs[:], pp[:],
                                             sin_sb[:, tcol:tcol + 1024])
                        nc.vector.stream_shuffle(rw[:], rs[:], SHUF_MASK)
                        nc.vector.tensor_add(dst[:, scol:scol + 1024],
                                             ra[:], rw[:])

            op_queue = []
            op_osb = {}

            def emit_outproj_tile(ob, mm):
                if (ob, mm // 4) not in op_osb:
                    op_osb[(ob, mm // 4)] = sb.tile(
                        [128, 4, D], BF16, name=f"osb{ob}_{mm // 4}",
                        tag="osb", bufs=2)
                osb = op_osb[(ob, mm // 4)]
                col = ob * T + mm * 128
                ops = ps.tile([128, 1024], F32, name=f"ops{ob}_{mm}",
                              tag="big", bufs=1)
                for hf in range(2):
                    nc.tensor.matmul(ops[:, hf * 512:(hf + 1) * 512],
                                     outT[:, :, col:col + 128],
                                     wo_sb[:, :, hf * 512:(hf + 1) * 512],
                                     start=True, stop=True, perf_mode=DR)
                if ob == 1 and mm >= 8:
                    nc.scalar.copy(osb[:, mm % 4, :], ops[:])
                else:
                    nc.vector.tensor_copy(osb[:, mm % 4, :], ops[:])
                if mm % 4 == 3:
                    tt = ob * 16 + (mm // 4) * 4
                    nc.sync.dma_start(out=partial[:, tt:tt + 4, :], in_=osb[:])

            def pop_op():
                if op_queue:
                    emit_outproj_tile(*op_queue.pop(0))

            def emit_attention_unit(u, op_b=None, self_feed=False):
                b, h = u // 2, u % 2
                hr = h * 64
                tb = b * T
                if op_b is not None and not self_feed:
                    op_queue.extend((op_b, mm) for mm in range(16))
                for s4 in range(NQB):
                    qc = tb + s4 * QB
                    av = ps.tile([128, QB], F32, name=f"av{u}_{s4}", tag="av",
                                 bufs=2)
                    npair = 2 * (s4 + 1)
                    for pr in range(npair):
                        sps = ps.tile([128, 2, QB], F32,
                                      name=f"sps{u}_{s4}_{pr}", tag="sps",
                                      bufs=2)
                        pT = sb.tile([128, 2, QB], FP8,
                                     name=f"pT{u}_{s4}_{pr}", tag="pT",
                                     bufs=4)
                        diag = []
                        for gi in range(2):
                            j = 2 * pr + gi
                            kc = tb + j * KC
                            off = j * KC - s4 * QB   # 0,128,256,384 on diag
                            if off >= 0:
                                if off > 0:
                                    nc.tensor.matmul(
                                        sps[:, gi, 0:off], negrow[0:1, 0:128],
                                        onesrow[0:1, 0:off],
                                        start=True, stop=True)
                                nc.tensor.matmul(
                                    sps[:, gi, off:QB],
                                    kT2[hr:hr + 64, kc:kc + KC],
                                    qT2[hr:hr + 64, qc + off:qc + QB],
                                    start=True, stop=True)
                                diag.append((gi, off))
                            else:
                                nc.tensor.matmul(
                                    sps[:, gi, :],
                                    kT2[hr:hr + 64, kc:kc + KC],
                                    qT2[hr:hr + 64, qc:qc + QB],
                                    start=True, stop=True)
                        nc.scalar.activation(pT[:], sps[:], EXP,
                                             scale=EXPSCALE)
                        for gi, off in diag:
                            nc.gpsimd.affine_select(
                                out=pT[:, gi, off:off + 128],
                                in_=pT[:, gi, off:off + 128],
                                compare_op=mybir.AluOpType.is_ge,
                                fill=0.0, base=0, pattern=[[1, 128]],
                                channel_multiplier=-1)
                        nc.tensor.matmul(av[:], vaug[u][:, 2 * pr:2 * pr + 2, :],
                                         pT[:], start=(pr == 0),
                                         stop=(pr == npair - 1), perf_mode=DR)
                        pop_op()
                    rD = sb.tile([64, QB], F32, name=f"rD{u}_{s4}", tag="rD",
                                 bufs=2)
                    nc.vector.reciprocal(rD[:], av[64:128, :])
                    nc.vector.tensor_mul(outT[:, h, qc:qc + QB],
                                         av[0:64, :], rD[:])
                    if self_feed:
                        op_queue.extend((b, s4 * 4 + mi) for mi in range(4))
                        pop_op()
                while self_feed and op_queue:
                    pop_op()

            emit_qkv_block(0)
            emit_qkv_block(1)
            emit_attention_unit(0)
            emit_qkv_block(2)
            emit_attention_unit(1)
            emit_qkv_block(3)
            emit_attention_unit(2, op_b=0)
            emit_attention_unit(3, op_b=1, self_feed=True)
            while op_queue:
                pop_op()

    nc.compile()
    return nc


def prep_in_maps(x, rope_freqs, w_qkv, w_out):
    import ml_dtypes
    NPBF = ml_dtypes.bfloat16
    NPF8 = ml_dtypes.float8_e4m3

    x = np.ascontiguousarray(np.asarray(x, np.float32))
    w_qkv = np.asarray(w_qkv, np.float32)
    w_out = np.asarray(w_out, np.float32)
    ang = np.asarray(rope_freqs, np.float64)   # (T, 32)

    # rope pair layout per head: quad q holds pairs 16q..16q+15 as
    # [16 evens | 16 odds]
    perm64 = np.concatenate([
        np.concatenate([np.arange(0, 32, 2) + 32 * q,
                        np.arange(1, 32, 2) + 32 * q])
        for q in range(2)])
    pair_of_p = np.concatenate([[16 * (p // 32) + (p % 16) for p in range(64)]
                                for _ in range(2)])          # 128 rows
    sign_of_p = np.array([1.0 if (p % 32) < 16 else -1.0 for p in range(128)])
    cos_t = np.cos(ang)[:, pair_of_p % 32].T                  # [128, T]
    sin_t = np.sin(ang)[:, pair_of_p % 32].T * sign_of_p[:, None]
    cosd = np.ascontiguousarray(cos_t.astype(NPBF))
    sind = np.ascontiguousarray(sin_t.astype(NPBF))

    xT = x.reshape(TOK, D).T                                  # [1024, 4096]
    xdr = np.ascontiguousarray(
        xT.reshape(8, 128, TOK).transpose(1, 0, 2).astype(NPF8))

    in_maps = []
    for c in range(NCORES):
        h0 = 2 * c
        qk_rows = np.concatenate([h0 * DH + perm64, (h0 + 1) * DH + perm64])
        v_rows = np.arange(h0 * DH, h0 * DH + 2 * DH)
        w_parts = []
        for base, rows in ((0, qk_rows), (D, qk_rows), (2 * D, v_rows)):
            w_parts.append(w_qkv[base + rows, :].T * WSCALE)  # [1024, 128]
        wcat = np.concatenate(w_parts, axis=1)                # [1024, 384]
        wall = np.ascontiguousarray(
            wcat.reshape(8, 128, 384).transpose(1, 0, 2).astype(NPF8))
        wodr = np.ascontiguousarray(
            (w_out[:, v_rows].T * WSCALE).reshape(2, 64, D)
            .transpose(1, 0, 2).astype(NPF8))
        in_maps.append({
            "xdr": xdr, "wall": wall, "wodr": wodr,
            "cosd": cosd, "sind": sind,
        })
    return in_maps


_CACHED = {}


def kernel(x, rope_freqs, w_qkv, w_out):
    from concourse.bass_utils import run_bass_kernel_spmd
    if "nc" not in _CACHED:
        _CACHED["nc"] = build_program()
    nc = _CACHED["nc"]
    in_maps = prep_in_maps(x, rope_freqs, w_qkv, w_out)
    res = run_bass_kernel_spmd(nc, in_maps, list(range(NCORES)))
    acc = np.zeros((128, 32, D), dtype=np.float32)
    for r in res.results:
        acc += np.asarray(r["partial"]).astype(np.float32)
    acc *= 1.0 / (WSCALE * WSCALE)
    out = acc.transpose(1, 0, 2).reshape(TOK, D)
    return np.ascontiguousarray(out.reshape(B, T, D))


# revision 7
# speedup vs baseline: 1.7885x; 1.0544x over previous
"""Causal self-attention TRN2 kernel v2: 8-way head-parallel, fp8 DoubleRow.

Per core c (heads h0=2c, h1=2c+1):
  - x, w_qkv (x16), w_out (x16) quantized to fp8e4m3 host-side; QKV projection
    runs fp8 DoubleRow matmuls (K=256/instr, 0.5 cyc/row).
  - RoPE pairs laid out [16 evens | 16 odds] per 32-partition quadrant so the
    complex-pair swap is a single DVE stream_shuffle (mask i^16).
  - Scores S^T[k,q] in bf16 (K=64); exp on ACT writes fp8 pT pairs; AV uses
    fp8 DoubleRow over chunk pairs with V_aug [v|ones] giving the softmax
    denominator in psum rows 64:127.
  - Causal: PE fills -1e7 left of the diagonal via K=1 matmuls; gpsimd
    affine_select triangles only the [128,128] diagonal block of pT.
  - out-proj: fp8 DR with outT_dr [64, 2(head), tok]; psum->sbuf bf16 copies
    split across DVE; batched bf16 DMA out (4 tok-tiles per DMA).
  - Host: sum 8 partials, scale 1/256, reshape.
"""

import sys

if "/opt/trn_rl_repo" not in sys.path:
    sys.path.insert(0, "/opt/trn_rl_repo")

import numpy as np

import concourse.bass as bass
import concourse.tile as tile
from concourse import bacc, mybir

F32 = mybir.dt.float32
BF16 = mybir.dt.bfloat16
FP8 = mybir.dt.float8e4
EXP = mybir.ActivationFunctionType.Exp
DR = mybir.MatmulPerfMode.DoubleRow

B, T, D, H, DH = 2, 2048, 1024, 16, 64
NCORES = 8
TOK = B * T
QB = 512
KC = 128
NQB = T // QB          # 4 q-blocks per unit
WSCALE = 16.0          # host scale on w_qkv / w_out before fp8
EXPSCALE = 0.125 / (WSCALE * WSCALE)
SHUF_MASK = [i ^ 16 for i in range(32)]


def build_program():
    nc = bacc.Bacc("TRN2", target_bir_lowering=False, debug=False,
                   num_devices=NCORES)
    xdr = nc.dram_tensor("xdr", [128, 8, TOK], FP8, kind="ExternalInput").ap()
    wall = nc.dram_tensor("wall", [128, 8, 384], FP8, kind="ExternalInput").ap()
    wodr = nc.dram_tensor("wodr", [64, 2, D], FP8, kind="ExternalInput").ap()
    cosd = nc.dram_tensor("cosd", [128, T], BF16, kind="ExternalInput").ap()
    sind = nc.dram_tensor("sind", [128, T], BF16, kind="ExternalInput").ap()
    partial = nc.dram_tensor("partial", [128, 32, D], BF16,
                             kind="ExternalOutput").ap()

    with tile.TileContext(nc) as tc:
        with tc.tile_pool(name="sb", bufs=1) as sb, \
             tc.tile_pool(name="ps", bufs=1, space="PSUM") as ps:
            # persistent SBUF
            w_sb = sb.tile([128, 8, 384], FP8, name="w_sb", tag="w_sb")
            wo_sb = sb.tile([64, 2, D], FP8, name="wo_sb", tag="wo_sb")
            cos_sb = sb.tile([128, T], BF16, name="cos_sb", tag="cos_sb")
            sin_sb = sb.tile([128, T], BF16, name="sin_sb", tag="sin_sb")
            qT2 = sb.tile([128, TOK], BF16, name="qT2", tag="qT2")
            kT2 = sb.tile([128, TOK], BF16, name="kT2", tag="kT2")
            outT = sb.tile([64, 2, TOK], FP8, name="outT", tag="outT")
            vaug = [sb.tile([128, 16, 128], FP8, name=f"vaug{u}", tag=f"vaug{u}")
                    for u in range(4)]
            identB = sb.tile([128, 64], BF16, name="identB", tag="identB")
            negrow = sb.tile([1, 512], BF16, name="negrow", tag="negrow")
            biasc = sb.tile([128, 1], F32, name="biasc", tag="biasc")
            onesrow = sb.tile([1, 512], BF16, name="onesrow", tag="onesrow")

            def emit_setup():
                nc.sync.dma_start(out=w_sb[:], in_=wall[:])
                nc.sync.dma_start(out=wo_sb[:], in_=wodr[:])
                nc.sync.dma_start(out=cos_sb[:], in_=cosd[:])
                nc.sync.dma_start(out=sin_sb[:], in_=sind[:])
                nc.gpsimd.memset(identB[:], 0.0)
                nc.gpsimd.affine_select(out=identB[:], in_=identB[:],
                                        compare_op=mybir.AluOpType.not_equal,
                                        fill=1.0, base=-64, pattern=[[-1, 64]],
                                        channel_multiplier=1)
                nc.gpsimd.memset(negrow[:], -1.0e7)
                nc.gpsimd.memset(biasc[:], -2.0)
                nc.gpsimd.memset(onesrow[:], 1.0)
                for u in range(4):
                    nc.gpsimd.memset(vaug[u][:, :, 64:128], 1.0)

            def emit_qkv_block(s):
                # block s: tokens [s*1024, (s+1)*1024); batch b = s // 2
                scol = s * 1024
                tcol = (s % 2) * 1024
                b = s // 2
                xts = sb.tile([128, 8, 1024], FP8, name=f"xts{s}", tag="xts",
                              bufs=2)
                nc.sync.dma_start(out=xts[:], in_=xdr[:, :, scol:scol + 1024])
                if s == 0:
                    emit_setup()
                for t, nm in ((0, "q"), (1, "k"), (2, "v")):
                    pp = ps.tile([128, 1024], F32, name=f"{nm}ps{s}",
                                 tag="big", bufs=1)
                    for hf in range(2):
                        for jj in range(4):
                            nc.tensor.matmul(
                                pp[:, hf * 512:(hf + 1) * 512],
                                w_sb[:, 2 * jj:2 * jj + 2,
                                     t * 128:(t + 1) * 128],
                                xts[:, 2 * jj:2 * jj + 2,
                                    hf * 512:(hf + 1) * 512],
                                start=(jj == 0), stop=(jj == 3), perf_mode=DR)
                    if nm == "v":
                        vraw = sb.tile([128, 1024], BF16, name=f"vraw{s}",
                                       tag="vraw", bufs=2)
                        nc.scalar.copy(vraw[:], pp[:])
                        for h in range(2):
                            u = b * 2 + h
                            idt = identB[0:64, :] if h == 0 else identB[64:128, :]
                            for g in range(2):
                                tp = ps.tile([128, 4, 64], BF16,
                                             name=f"tp{s}_{h}_{g}", tag="av",
                                             bufs=2)
                                for pi in range(4):
                                    tch = g * 4 + pi
                                    nc.tensor.transpose(
                                        tp[:, pi, :],
                                        vraw[h * 64:(h + 1) * 64,
                                             tch * 128:(tch + 1) * 128],
                                        idt)
                                cj = (s % 2) * 8 + g * 4
                                nc.scalar.copy(
                                    vaug[u][:, cj:cj + 4, 0:64], tp[:])
                    else:
                        dst = qT2 if nm == "q" else kT2
                        ra = sb.tile([128, 1024], BF16, name=f"ra{nm}{s}",
                                     tag="ra", bufs=2)
                        rs = sb.tile([128, 1024], BF16, name=f"rs{nm}{s}",
                                     tag="rs", bufs=2)
                        rw = sb.tile([128, 1024], BF16, name=f"rw{nm}{s}",
                                     tag="rw", bufs=2)
                        nc.vector.tensor_mul(ra[:], pp[:],
                                             cos_sb[:, tcol:tcol + 1024])
                        nc.vector.tensor_mul(rs[:], pp[:],
                                             sin_sb[:, tcol:tcol + 1024])
                        nc.vector.stream_shuffle(rw[:], rs[:], SHUF_MASK)
                        nc.vector.tensor_add(dst[:, scol:scol + 1024],
                                             ra[:], rw[:])

            op_queue = []
            op_osb = {}

            def emit_outproj_tile(ob, mm):
                if (ob, mm // 4) not in op_osb:
                    op_osb[(ob, mm // 4)] = sb.tile(
                        [128, 4, D], BF16, name=f"osb{ob}_{mm // 4}",
                        tag="osb", bufs=2)
                osb = op_osb[(ob, mm // 4)]
                col = ob * T + mm * 128
                ops = ps.tile([128, 1024], F32, name=f"ops{ob}_{mm}",
                              tag="big", bufs=1)
                for hf in range(2):
                    nc.tensor.matmul(ops[:, hf * 512:(hf + 1) * 512],
                                     outT[:, :, col:col + 128],
                                     wo_sb[:, :, hf * 512:(hf + 1) * 512],
                                     start=True, stop=True, perf_mode=DR)
                if ob == 1 and mm >= 8:
                    nc.scalar.copy(osb[:, mm % 4, :], ops[:])
                else:
                    nc.vector.tensor_copy(osb[:, mm % 4, :], ops[:])
                if mm % 4 == 3:
                    tt = ob * 16 + (mm // 4) * 4
                    nc.sync.dma_start(out=partial[:, tt:tt + 4, :], in_=osb[:])

            def pop_op():
                if op_queue:
                    emit_outproj_tile(*op_queue.pop(0))

            def emit_attention_unit(u, op_b=None, self_feed=False):
                b, h = u // 2, u % 2
                hr = h * 64
                tb = b * T
                if op_b is not None and not self_feed:
                    op_queue.extend((op_b, mm) for mm in range(16))
                for s4 in range(NQB):
                    qc = tb + s4 * QB
                    av = ps.tile([128, QB], F32, name=f"av{u}_{s4}", tag="av",
                                 bufs=2)
                    npair = 2 * (s4 + 1)
                    for pr in range(npair):
                        sps = ps.tile([128, 2, QB], F32,
                                      name=f"sps{u}_{s4}_{pr}", tag="sps",
                                      bufs=2)
                        pT = sb.tile([128, 2, QB], FP8,
                                     name=f"pT{u}_{s4}_{pr}", tag="pT",
                                     bufs=4)
                        diag = []
                        for gi in range(2):
                            j = 2 * pr + gi
                            kc = tb + j * KC
                            off = j * KC - s4 * QB   # 0,128,256,384 on diag
                            if off >= 0:
                                if off > 0:
                                    nc.tensor.matmul(
                                        sps[:, gi, 0:off], negrow[0:1, 0:128],
                                        onesrow[0:1, 0:off],
                                        start=True, stop=True)
                                nc.tensor.matmul(
                                    sps[:, gi, off:QB],
                                    kT2[hr:hr + 64, kc:kc + KC],
                                    qT2[hr:hr + 64, qc + off:qc + QB],
                                    start=True, stop=True)
                                diag.append((gi, off))
                            else:
                                nc.tensor.matmul(
                                    sps[:, gi, :],
                                    kT2[hr:hr + 64, kc:kc + KC],
                                    qT2[hr:hr + 64, qc:qc + QB],
                                    start=True, stop=True)
                        nc.scalar.activation(pT[:], sps[:], EXP,
                                             bias=biasc[:], scale=EXPSCALE)
                        for gi, off in diag:
                            nc.gpsimd.affine_select(
                                out=pT[:, gi, off:off + 128],
                                in_=pT[:, gi, off:off + 128],
                                compare_op=mybir.AluOpType.is_ge,
                                fill=0.0, base=0, pattern=[[1, 128]],
                                channel_multiplier=-1)
                        nc.tensor.matmul(av[:], vaug[u][:, 2 * pr:2 * pr + 2, :],
                                         pT[:], start=(pr == 0),
                                         stop=(pr == npair - 1), perf_mode=DR)
                        pop_op()
                    rD = sb.tile([64, QB], F32, name=f"rD{u}_{s4}", tag="rD",
                                 bufs=2)
                    nc.vector.reciprocal(rD[:], av[64:128, :])
                    nc.vector.tensor_mul(outT[:, h, qc:qc + QB],
                                         av[0:64, :], rD[:])
                    if self_feed:
                        op_queue.extend((b, s4 * 4 + mi) for mi in range(4))
                        pop_op()
                while self_feed and op_queue:
                    pop_op()

            emit_qkv_block(0)
            emit_qkv_block(1)
            emit_attention_unit(0)
            emit_qkv_block(2)
            emit_attention_unit(1)
            emit_qkv_block(3)
            emit_attention_unit(2, op_b=0)
            emit_attention_unit(3, op_b=1, self_feed=True)
            while op_queue:
                pop_op()

    nc.compile()
    return nc


def prep_in_maps(x, rope_freqs, w_qkv, w_out):
    import ml_dtypes
    NPBF = ml_dtypes.bfloat16
    NPF8 = ml_dtypes.float8_e4m3

    x = np.ascontiguousarray(np.asarray(x, np.float32))
    w_qkv = np.asarray(w_qkv, np.float32)
    w_out = np.asarray(w_out, np.float32)
    ang = np.asarray(rope_freqs, np.float64)   # (T, 32)

    # rope pair layout per head: quad q holds pairs 16q..16q+15 as
    # [16 evens | 16 odds]
    perm64 = np.concatenate([
        np.concatenate([np.arange(0, 32, 2) + 32 * q,
                        np.arange(1, 32, 2) + 32 * q])
        for q in range(2)])
    pair_of_p = np.concatenate([[16 * (p // 32) + (p % 16) for p in range(64)]
                                for _ in range(2)])          # 128 rows
    sign_of_p = np.array([1.0 if (p % 32) < 16 else -1.0 for p in range(128)])
    cos_t = np.cos(ang)[:, pair_of_p % 32].T                  # [128, T]
    sin_t = np.sin(ang)[:, pair_of_p % 32].T * sign_of_p[:, None]
    cosd = np.ascontiguousarray(cos_t.astype(NPBF))
    sind = np.ascontiguousarray(sin_t.astype(NPBF))

    xT = x.reshape(TOK, D).T                                  # [1024, 4096]
    xdr = np.ascontiguousarray(
        xT.reshape(8, 128, TOK).transpose(1, 0, 2).astype(NPF8))

    in_maps = []
    for c in range(NCORES):
        h0 = 2 * c
        qk_rows = np.concatenate([h0 * DH + perm64, (h0 + 1) * DH + perm64])
        v_rows = np.arange(h0 * DH, h0 * DH + 2 * DH)
        w_parts = []
        for base, rows in ((0, qk_rows), (D, qk_rows), (2 * D, v_rows)):
            w_parts.append(w_qkv[base + rows, :].T * WSCALE)  # [1024, 128]
        wcat = np.concatenate(w_parts, axis=1)                # [1024, 384]
        wall = np.ascontiguousarray(
            wcat.reshape(8, 128, 384).transpose(1, 0, 2).astype(NPF8))
        wodr = np.ascontiguousarray(
            (w_out[:, v_rows].T * WSCALE).reshape(2, 64, D)
            .transpose(1, 0, 2).astype(NPF8))
        in_maps.append({
            "xdr": xdr, "wall": wall, "wodr": wodr,
            "cosd": cosd, "sind": sind,
        })
    return in_maps


_CACHED = {}


def kernel(x, rope_freqs, w_qkv, w_out):
    from concourse.bass_utils import run_bass_kernel_spmd
    if "nc" not in _CACHED:
        _CACHED["nc"] = build_program()
    nc = _CACHED["nc"]
    in_maps = prep_in_maps(x, rope_freqs, w_qkv, w_out)
    res = run_bass_kernel_spmd(nc, in_maps, list(range(NCORES)))
    acc = np.zeros((128, 32, D), dtype=np.float32)
    for r in res.results:
        acc += np.asarray(r["partial"]).astype(np.float32)
    acc *= 1.0 / (WSCALE * WSCALE)
    out = acc.transpose(1, 0, 2).reshape(TOK, D)
    return np.ascontiguousarray(out.reshape(B, T, D))
